# revision 30
# baseline (speedup 1.0000x reference)
"""Trainium2 Bass kernel for a pre-LN transformer block (B=4,S=1024,E=1024,H=16).

Sharding: 8 cores = (batch b, parity p). Core (b,p) computes the full block for
batch b restricted to query tokens {q : q % 2 == p} (512 tokens). K/V are
computed for all 1024 tokens of the batch (duplicated across the pair).

Local token order: tile t in 0..3 holds global queries 256*(3-t)+2j+p, so the
causal context per tile is 1024-256t.

QKV projections run in fp8(e4m3) DoubleRow mode (weights host-scaled by 64,
descaled via the bias-activation scale). Attention is q-tile-outer with the
head loop inner; att_weights accumulate in PSUM via pass-through matmuls
against a host-supplied I/16 identity; P^T for the AV matmul comes from one
batched DMA transpose per (tile, 8-head group). Work for tile t-1 (aw, AV,
out_proj) interleaves into tile t's head loop to keep the PE warm.
"""

import math
import os

import numpy as np

B, S, E, H = 4, 1024, 1024, 16
DH = E // H
FF = 4 * E
P = 128
NCORES = 8
LN_EPS = 1e-5
NEG = -30000.0
W8 = 64.0
IW8 = 1.0 / W8

_BUILD_CACHE = {}
LAST_RESULTS = None


def _build():
    if "nc" in _BUILD_CACHE:
        return _BUILD_CACHE["nc"]

    import concourse.bacc as bacc
    import concourse.bass as bass
    import concourse.mybir as mybir
    import concourse.tile as tile

    f32 = mybir.dt.float32
    bf16 = mybir.dt.bfloat16
    f8 = mybir.dt.float8e4
    AF = mybir.ActivationFunctionType
    ALU = mybir.AluOpType
    DR = mybir.MatmulPerfMode.DoubleRow

    nc = bacc.Bacc(
        "TRN2",
        target_bir_lowering=False,
        debug=False,
        enable_asserts=False,
        num_devices=NCORES,
    )

    # ---- DRAM I/O ----
    xg_d = nc.dram_tensor("x_glob_bf", [P, 8 * E], bf16, kind="ExternalInput")
    xrb_d = nc.dram_tensor("x_res_bf", [P, 4 * E], bf16, kind="ExternalInput")
    wq_d = nc.dram_tensor("wq_sb", [P, 8, E], f8, kind="ExternalInput")
    wk_d = nc.dram_tensor("wk_sb", [P, 8, E], f8, kind="ExternalInput")
    wv_d = nc.dram_tensor("wv_sb", [P, 8, E], f8, kind="ExternalInput")
    wo_d = nc.dram_tensor("wo_sb", [P, 8, E], bf16, kind="ExternalInput")
    wf_d = nc.dram_tensor("wf_sb", [8, P, 4, 8, P], bf16, kind="ExternalInput")
    wpa_d = nc.dram_tensor("wpa_sb", [8, P, 4, 512], bf16, kind="ExternalInput")
    wpb_d = nc.dram_tensor("wpb_sb", [8, P, 4, 512], bf16, kind="ExternalInput")
    bq_d = nc.dram_tensor("bq_sb", [P, 8], f32, kind="ExternalInput")
    bk_d = nc.dram_tensor("bk_sb", [P, 8], f32, kind="ExternalInput")
    bfc_d = nc.dram_tensor("bfc_sb", [P, 32], f32, kind="ExternalInput")
    bvec2_d = nc.dram_tensor("bvec2", [E], f32, kind="ExternalInput")
    msk_d = nc.dram_tensor("msk", [P, 4, 256], bf16, kind="ExternalInput")
    yout_d = nc.dram_tensor("y_out", [S // 2, E], bf16, kind="ExternalOutput")
    awout_d = nc.dram_tensor("aw_out", [S // 2, S], bf16, kind="ExternalOutput")

    xg = xg_d.ap()
    yout = yout_d.ap()
    awout = awout_d.ap()

    def bcast_ap(handle):
        ap = handle.ap()
        return bass.AP(tensor=ap.tensor, offset=ap.offset, ap=[[0, P]] + list(ap.ap))

    with tile.TileContext(nc) as tc:
        import contextlib

        stack = contextlib.ExitStack()
        with stack:
            persist = stack.enter_context(tc.tile_pool(name="persist", bufs=1))

            xres = persist.tile([P, 4, E], bf16, tag="xres")
            ysb = persist.tile([P, 4, E], f32, tag="ysb")
            z2T = persist.tile([P, 8, 512], bf16, tag="z2T")
            bvec2b = persist.tile([P, E], f32, tag="bvec2b")
            mskb = persist.tile([P, 4, 256], bf16, tag="mskb")
            bqsb = persist.tile([P, 8], f32, tag="bqsb")
            bksb = persist.tile([P, 8], f32, tag="bksb")
            bfcsb = persist.tile([P, 32], f32, tag="bfcsb")
            epsb = persist.tile([P, 1], f32, tag="epsb")
            warm = persist.tile([P, 64], bf16, tag="warm")

            # --- PE warmup: keep HAM at K=8/8 until the first QKV matmul.
            nc.vector.memset(epsb[:], LN_EPS)
            nc.vector.memset(warm[:], 0.0)
            with tc.tile_pool(name="warmp", bufs=1, space="PSUM") as wpp:
                wps = wpp.tile([64, 64], f32, tag="wps")
                for _ in range(20):
                    nc.tensor.matmul(
                        wps[:], lhsT=warm[:], rhs=warm[:], start=True, stop=True
                    )

            import contextlib as _ctl
            astack = _ctl.ExitStack()
            apool = astack.enter_context(tc.tile_pool(name="attnbuf", bufs=1))
            hstack = _ctl.ExitStack()
            hpool = hstack.enter_context(tc.tile_pool(name="hbuf", bufs=1))
            hT = hpool.tile([P, 8, S], bf16, tag="hT")
            hT8 = apool.tile([P, 8, S], f8, tag="hT8")
            kdt = apool.tile([P, 8, S], bf16, tag="kdt")
            vtd = apool.tile([P, 8, E], bf16, tag="vtd")
            qdt = apool.tile([P, 8, 512], bf16, tag="qdt")
            attsb = apool.tile([P, 8, 512], bf16, tag="attsb")

            # QKV weight pool opened early so transfers overlap LayerNorm
            wstack = _ctl.ExitStack()
            wp = wstack.enter_context(tc.tile_pool(name="wqkv", bufs=3))
            wq_sb = wp.tile([P, 8, E], f8, tag="w", name="wq_sb")
            wk_sb = wp.tile([P, 8, E], f8, tag="w", name="wk_sb")

            # ---------- Phase A: LayerNorm1 (global tiles only) ----------
            # Local (query) tokens are a stride-2 subset of the pair-permuted
            # global order, so Q reads hT8 directly -- no separate local LN.
            xga = hpool.tile([P, 8, E], bf16, tag="xga")
            with tc.tile_pool(name="lnp", bufs=4) as lnp, tc.tile_pool(
                name="lns", bufs=8
            ) as lns:

                # priority order: K-ch0 needs xga[0:4] + wk first; late-needed
                # wq/xga[4:6] ride the slow gpsimd queue
                nc.sync.dma_start(out=xga[:, 0:2, :], in_=xg_d.ap()[:, : 2 * E])
                nc.scalar.dma_start(
                    out=xga[:, 2:4, :], in_=xg_d.ap()[:, 2 * E : 4 * E]
                )
                nc.sync.dma_start(out=wk_sb[:], in_=wk_d.ap())
                nc.sync.dma_start(out=bksb[:], in_=bk_d.ap())
                nc.sync.dma_start(out=bqsb[:], in_=bq_d.ap())
                nc.gpsimd.dma_start(
                    out=xga[:, 4:6, :], in_=xg_d.ap()[:, 4 * E : 6 * E]
                )
                nc.scalar.dma_start(out=xga[:, 6:8, :], in_=xg_d.ap()[:, 6 * E :])
                nc.gpsimd.dma_start(out=wq_sb[:], in_=wq_d.ap())
                wv_sb = apool.tile([P, 8, E], f8, tag="wv_sb")
                nc.scalar.dma_start(out=wv_sb[:], in_=wv_d.ap())
                # Preload the Gelu table off the critical path.
                gel_warm = persist.tile([P, 1], f32, tag="gel_warm")
                nc.scalar.activation(
                    gel_warm[:], epsb[:], AF.Gelu_apprx_tanh, bias=epsb[:], scale=1.0
                )

                def ln_tile(idx, xt, dst, dst8):
                    st = lns.tile([P, 2, 6], f32, tag="ln_st")
                    nc.vector.bn_stats(out=st[:, 0, :], in_=xt[:, 0:512])
                    nc.vector.bn_stats(out=st[:, 1, :], in_=xt[:, 512:1024])
                    mv = lns.tile([P, 2], f32, tag="ln_mv")
                    nc.vector.bn_aggr(out=mv[:], in_=st[:])
                    sd = lns.tile([P, 1], f32, tag="ln_sd")
                    nc.scalar.activation(
                        sd[:], mv[:, 1:2], AF.Sqrt, bias=epsb[:], scale=1.0
                    )
                    rs = lns.tile([P, 1], f32, tag="ln_rs")
                    nc.vector.reciprocal(rs[:], sd[:])
                    nmu = lns.tile([P, 1], f32, tag="ln_nmu")
                    nc.vector.tensor_scalar(
                        out=nmu[:], in0=mv[:, 0:1], scalar1=rs[:], scalar2=-1.0,
                        op0=ALU.mult, op1=ALU.mult,
                    )
                    zt = lnp.tile([P, E], bf16, tag="ln_z")
                    nc.scalar.activation(
                        zt[:], xt[:], AF.Identity, bias=nmu[:], scale=rs[:]
                    )
                    nc.sync.dma_start_transpose(out=dst, in_=zt[:])
                    if idx % 2 == 0:
                        nc.vector.tensor_copy(dst8, dst)
                    else:
                        nc.scalar.copy(dst8, dst)

                for t in range(8):
                    ln_tile(
                        t + 1,
                        xga[:, t, :],
                        hT[:, :, P * t : P * (t + 1)],
                        hT8[:, :, P * t : P * (t + 1)],
                    )

            # ---------- Phase B: QKV projections (fp8 DoubleRow) ----------
            with tc.tile_pool(name="psqkv", bufs=4, space="PSUM") as pp:
                def emit_k(ch):
                    for oc in range(8):
                        ps = pp.tile([P, 512], f32, tag="ps")
                        for kp in range(4):
                            nc.tensor.matmul(
                                ps[:],
                                lhsT=wk_sb[:, 2 * kp : 2 * kp + 2, P * oc : P * (oc + 1)],
                                rhs=hT8[:, 2 * kp : 2 * kp + 2, 512 * ch : 512 * (ch + 1)],
                                start=(kp == 0),
                                stop=(kp == 3),
                                perf_mode=DR,
                            )
                        nc.scalar.activation(
                            kdt[:, oc, 512 * ch : 512 * (ch + 1)],
                            ps[:],
                            AF.Identity,
                            bias=bksb[:, oc : oc + 1],
                            scale=IW8,
                        )

                emit_k(0)
                # Q: local tokens are the even columns of the permuted hT8
                for oc in range(8):
                    ps = pp.tile([P, 512], f32, tag="ps")
                    for kp in range(4):
                        nc.tensor.matmul(
                            ps[:],
                            lhsT=wq_sb[:, 2 * kp : 2 * kp + 2, P * oc : P * (oc + 1)],
                            rhs=hT8[:, 2 * kp : 2 * kp + 2, 0 : S : 2],
                            start=(kp == 0),
                            stop=(kp == 3),
                            perf_mode=DR,
                        )
                    nc.scalar.activation(
                        qdt[:, oc, :],
                        ps[:],
                        AF.Identity,
                        bias=bqsb[:, oc : oc + 1],
                        scale=1.0 / (W8 * 8.0),
                    )
                emit_k(1)
                # constants needed by attention / residual path
                nc.gpsimd.dma_start(out=mskb[:], in_=msk_d.ap())
                nc.gpsimd.dma_start(out=xres[:], in_=xrb_d.ap())

            wstack.close()
            hstack.close()

            # ---------- Phase C: attention, q-tile-outer ----------
            with tc.tile_pool(name="wo", bufs=1) as wop, tc.tile_pool(
                name="pbp", bufs=1
            ) as pbp, tc.tile_pool(name="ptp", bufs=2) as ptp, tc.tile_pool(
                name="awacc", bufs=2
            ) as awp, tc.tile_pool(name="awpr", bufs=2) as awpp, tc.tile_pool(
                name="psc", bufs=2, space="PSUM"
            ) as scp, tc.tile_pool(
                name="psav", bufs=2, space="PSUM"
            ) as avp, tc.tile_pool(
                name="pso", bufs=1, space="PSUM"
            ) as pop, tc.tile_pool(
                name="psv", bufs=1, space="PSUM"
            ) as vpp, tc.tile_pool(name="asm", bufs=10) as smp, tc.tile_pool(
                name="ln2s", bufs=2
            ) as lns2, tc.tile_pool(name="ln2p", bufs=2) as lnp2:

                def emit_v(tkt):
                    for ch in range(2):
                        vps = vpp.tile([P, 512], f32, tag="vps")
                        for kp in range(4):
                            nc.tensor.matmul(
                                vps[:],
                                lhsT=hT8[:, 2 * kp : 2 * kp + 2, P * tkt : P * (tkt + 1)],
                                rhs=wv_sb[:, 2 * kp : 2 * kp + 2, 512 * ch : 512 * (ch + 1)],
                                start=(kp == 0),
                                stop=(kp == 3),
                                perf_mode=DR,
                            )
                        nc.vector.tensor_scalar(
                            out=vtd[:, tkt, 512 * ch : 512 * (ch + 1)],
                            in0=vps[:],
                            scalar1=IW8,
                            scalar2=None,
                            op0=ALU.mult,
                        )

                wo_sb = wop.tile([P, 8, E], bf16, tag="wo")
                nc.gpsimd.dma_start(out=wo_sb[:], in_=wo_d.ap())
                nc.gpsimd.dma_start(out=bvec2b[:], in_=bcast_ap(bvec2_d))

                CTX = [S - 256 * t for t in range(4)]
                pb_t = {}
                pT_t = {}

                def emit_scores(tl, h, sc):
                    ctx = CTX[tl]
                    i, s = h // 2, h % 2
                    for c0 in range(0, ctx, 512):
                        n = min(512, ctx - c0)
                        last = c0 + n == ctx
                        nc.tensor.matmul(
                            sc[:, c0 : c0 + n],
                            lhsT=qdt[64 * s : 64 * (s + 1), i, P * (3 - tl) : P * (4 - tl)],
                            rhs=kdt[64 * s : 64 * (s + 1), i, c0 : c0 + n],
                            start=True,
                            stop=last,
                            skip_group_check=True,
                        )
                    # additive causal mask on the boundary slab (vector TT,
                    # off the PE critical path)
                    nc.vector.tensor_add(
                        sc[:, ctx - 256 : ctx],
                        sc[:, ctx - 256 : ctx],
                        mskb[:, tl, :],
                    )

                def emit_post(tl, h, sc):
                    ctx = CTX[tl]
                    g, hh = h // 4, h % 4
                    pbt = pb_t[(tl, g)]
                    sl_lo, sl_hi = hh * ctx, (hh + 1) * ctx
                    pf = smp.tile([P, S], bf16, tag="pf", bufs=4)
                    den = smp.tile([P, 1], f32, tag="den")
                    nc.scalar.activation(
                        pf[:, :ctx], sc[:, :ctx], AF.Exp, accum_out=den[:]
                    )
                    r = smp.tile([P, 1], f32, tag="r")
                    nc.vector.reciprocal(r[:], den[:])
                    if h % 4 == 1:
                        nc.scalar.activation(
                            pbt[:, sl_lo:sl_hi], pf[:, :ctx], AF.Copy, scale=r[:]
                        )
                    else:
                        nc.vector.tensor_scalar(
                            out=pbt[:, sl_lo:sl_hi],
                            in0=pf[:, :ctx],
                            scalar1=r[:],
                            scalar2=None,
                            op0=ALU.mult,
                        )
                    # att_weights: head-pair partial sums on vector, chained
                    # into awacc on gpsimd (both bf16; host rescales by 1/16)
                    awacc = aw_t[tl]
                    if h % 2 == 1:
                        lo_prev = (h - 1) % 4 * ctx
                        if h == 1:
                            nc.gpsimd.tensor_add(
                                awacc[:, :ctx],
                                pbt[:, lo_prev : lo_prev + ctx],
                                pbt[:, sl_lo:sl_hi],
                            )
                        else:
                            nc.gpsimd.tensor_add(
                                awacc[:, :ctx],
                                awacc[:, :ctx],
                                pbt[:, lo_prev : lo_prev + ctx],
                            )
                            nc.gpsimd.tensor_add(
                                awacc[:, :ctx],
                                awacc[:, :ctx],
                                pbt[:, sl_lo:sl_hi],
                            )
                    if h == 15:
                        nc.gpsimd.dma_start(
                            out=awout[P * tl : P * (tl + 1), 0:ctx],
                            in_=awacc[:, :ctx],
                        )
                    if hh == 3:
                        pT = ptp.tile(
                            [P, 32, P], bf16, tag=f"ptg{g % 2}", name=f"pt{tl}_{g}"
                        )
                        pT_t[(tl, g)] = pT
                        ([nc.sync, nc.scalar][g % 2]).dma_start_transpose(
                            out=pT[:, : 4 * (ctx // P), :], in_=pbt[:, : 4 * ctx]
                        )

                def emit_av(tl, i):
                    ctx = CTX[tl]
                    nkc = ctx // P
                    avps = avp.tile([P, P], f32, tag="av")
                    for kc in range(nkc):
                        for sub in range(2):
                            h = 2 * i + sub
                            g, hh = h // 4, h % 4
                            nc.tensor.matmul(
                                avps[64 * sub : 64 * (sub + 1), :],
                                lhsT=vtd[:, kc, 64 * h : 64 * (h + 1)],
                                rhs=pT_t[(tl, g)][:, hh * nkc + kc, :],
                                start=(kc == 0),
                                stop=(kc == nkc - 1),
                                skip_group_check=True,
                            )
                    if i % 2 == 0:
                        nc.vector.tensor_copy(
                            attsb[:, i, P * tl : P * (tl + 1)], avps[:]
                        )
                    else:
                        nc.scalar.copy(attsb[:, i, P * tl : P * (tl + 1)], avps[:])

                def emit_pd(m):
                    # out_proj + residual + LayerNorm2 for query tile m
                    for o2 in range(2):
                        ps = pop.tile([P, 512], f32, tag="po")
                        for i in range(8):
                            nc.tensor.matmul(
                                ps[:],
                                lhsT=attsb[:, i, P * m : P * (m + 1)],
                                rhs=wo_sb[:, i, 512 * o2 : 512 * (o2 + 1)],
                                start=(i == 0),
                                stop=(i == 7),
                            )
                        nc.vector.tensor_add(
                            ysb[:, m, 512 * o2 : 512 * (o2 + 1)],
                            ps[:],
                            xres[:, m, 512 * o2 : 512 * (o2 + 1)],
                        )
                    st = lns2.tile([P, 2, 6], f32, tag="l2st")
                    nc.vector.bn_stats(out=st[:, 0, :], in_=ysb[:, m, 0:512])
                    nc.vector.bn_stats(out=st[:, 1, :], in_=ysb[:, m, 512:1024])
                    mv = lns2.tile([P, 2], f32, tag="l2mv")
                    nc.vector.bn_aggr(out=mv[:], in_=st[:])
                    sd = lns2.tile([P, 1], f32, tag="l2sd")
                    nc.scalar.activation(
                        sd[:], mv[:, 1:2], AF.Sqrt, bias=epsb[:], scale=1.0
                    )
                    rs = lns2.tile([P, 1], f32, tag="l2rs")
                    nc.vector.reciprocal(rs[:], sd[:])
                    nmu = lns2.tile([P, 1], f32, tag="l2nmu")
                    nc.vector.tensor_scalar(
                        out=nmu[:], in0=mv[:, 0:1], scalar1=rs[:], scalar2=-1.0,
                        op0=ALU.mult, op1=ALU.mult,
                    )
                    z2 = lnp2.tile([P, E], bf16, tag="l2z")
                    nc.scalar.activation(
                        z2[:], ysb[:, m, :], AF.Identity, bias=nmu[:], scale=rs[:]
                    )
                    nc.sync.dma_start_transpose(
                        out=z2T[:, :, P * m : P * (m + 1)], in_=z2[:]
                    )
                    nc.gpsimd.tensor_add(ysb[:, m, :], ysb[:, m, :], bvec2b[:])

                aw_t = {}
                for tl in range(4):
                    for g in range(4):
                        pb_t[(tl, g)] = pbp.tile(
                            [P, 4 * S], bf16, tag=f"pbg{g % 2}", name=f"pb{tl}_{g}"
                        )
                    aw_t[tl] = awp.tile([P, S], bf16, tag="awacc", name=f"awacc{tl}")
                    prev = None
                    for h in range(16):
                        sc = scp.tile([P, S], f32, tag="sc")
                        emit_scores(tl, h, sc)
                        if tl == 0 and h % 2 == 0:
                            emit_v(h // 2)
                        if tl >= 1 and h % 2 == 0:
                            emit_av(tl - 1, h // 2)
                        if prev is not None:
                            emit_post(tl, *prev)
                        prev = (h, sc)
                    emit_post(tl, *prev)
                    if tl >= 1:
                        emit_pd(tl - 1)
                # tail: prefetch first MLP weight block, tile 3's AV + out_proj
                pre_bf4 = persist.tile([P, 4, 8, P], bf16, tag="pre_bf4")
                nc.scalar.dma_start(out=pre_bf4[:], in_=wf_d.ap()[0])
                pre_bpa4 = persist.tile([P, 4, 512], bf16, tag="pre_bpa4")
                nc.gpsimd.dma_start(out=pre_bpa4[:], in_=wpa_d.ap()[0])
                nc.scalar.dma_start(out=bfcsb[:], in_=bfc_d.ap())
                for i in range(8):
                    emit_av(3, i)
                emit_pd(3)

            astack.close()

            # ---------- Phase F/G: MLP (bf16) ----------
            with tc.tile_pool(name="mlp", bufs=1) as mp, tc.tile_pool(
                name="blk", bufs=3
            ) as bp, tc.tile_pool(name="x3p", bufs=3) as xp, tc.tile_pool(
                name="pspj", bufs=1, space="PSUM"
            ) as jp:
                hidT = mp.tile([P, 32, 512], bf16, tag="hidT")
                psA = [
                    jp.tile([P, 512], f32, tag=f"pja{m}", name=f"pja{m}")
                    for m in range(4)
                ]
                wfap = wf_d.ap()
                wpaap = wpa_d.ap()
                wpbap = wpb_d.ap()

                def emit_pja(hc, bpa):
                    hi = hc % 4
                    for m in range(4):
                        nc.tensor.matmul(
                            psA[m][:],
                            lhsT=hidT[:, hc, P * m : P * (m + 1)],
                            rhs=bpa[:, hi, :],
                            start=(hc == 0),
                            stop=(hc == 31),
                        )

                with tc.tile_pool(name="psf", bufs=4, space="PSUM") as fp:
                    bf_cur, bpa_cur = pre_bf4, pre_bpa4
                    pend = None  # (hc, bpa tile)
                    for hg in range(8):
                        if hg < 7:
                            bf_nxt = bp.tile([P, 4, 8, P], bf16, tag="bf4")
                            ([nc.scalar, nc.sync][hg % 2]).dma_start(
                                out=bf_nxt[:], in_=wfap[hg + 1]
                            )
                            bpa_nxt = bp.tile([P, 4, 512], bf16, tag="bpa4")
                            ([nc.sync, nc.scalar][hg % 2]).dma_start(
                                out=bpa_nxt[:], in_=wpaap[hg + 1]
                            )
                        if hg == 0:
                            # column-split: tiles 0-2 of z2T are ready well
                            # before tile 3, so run 3/4-width fc first.
                            psFs = [
                                fp.tile([P, 512], f32, tag="psF", name=f"psF0_{i}")
                                for i in range(4)
                            ]
                            for c0, c1 in ((0, 384), (384, 512)):
                                for hi in range(4):
                                    for kc in range(8):
                                        nc.tensor.matmul(
                                            psFs[hi][:, c0:c1],
                                            lhsT=bf_cur[:, hi, kc, :],
                                            rhs=z2T[:, kc, c0:c1],
                                            start=(kc == 0),
                                            stop=(kc == 7),
                                        )
                                    nc.scalar.activation(
                                        hidT[:, hi, c0:c1],
                                        psFs[hi][:, c0:c1],
                                        AF.Gelu_apprx_tanh,
                                        bias=bfcsb[:, hi : hi + 1],
                                        scale=1.0,
                                    )
                            emit_pja(0, bpa_cur)
                            emit_pja(1, bpa_cur)
                            emit_pja(2, bpa_cur)
                            pend = (3, bpa_cur)
                        else:
                            for hi in range(4):
                                hc = 4 * hg + hi
                                psF = fp.tile([P, 512], f32, tag="psF")
                                for kc in range(8):
                                    nc.tensor.matmul(
                                        psF[:],
                                        lhsT=bf_cur[:, hi, kc, :],
                                        rhs=z2T[:, kc, :],
                                        start=(kc == 0),
                                        stop=(kc == 7),
                                    )
                                nc.scalar.activation(
                                    hidT[:, hc, :],
                                    psF[:],
                                    AF.Gelu_apprx_tanh,
                                    bias=bfcsb[:, hc : hc + 1],
                                    scale=1.0,
                                )
                                if pend is not None:
                                    emit_pja(*pend)
                                pend = (hc, bpa_cur)
                        if hg < 7:
                            bf_cur, bpa_cur = bf_nxt, bpa_nxt
                    emit_pja(*pend)
                    for m in range(4):
                        x3 = xp.tile([P, 512], bf16, tag="x3")
                        nc.vector.tensor_add(x3[:], psA[m][:], ysb[:, m, 0:512])
                        nc.sync.dma_start(
                            out=yout[P * m : P * (m + 1), 0:512], in_=x3[:]
                        )
                with tc.tile_pool(name="pspjb", bufs=1, space="PSUM") as jpb:
                    psB = [
                        jpb.tile([P, 512], f32, tag=f"pjb{m}", name=f"pjb{m}")
                        for m in range(4)
                    ]
                    for hg in range(8):
                        bpb4 = bp.tile([P, 4, 512], bf16, tag="bpb4")
                        ([nc.scalar, nc.sync][hg % 2]).dma_start(
                            out=bpb4[:], in_=wpbap[hg]
                        )
                        for hi in range(4):
                            hc = 4 * hg + hi
                            for m in range(4):
                                nc.tensor.matmul(
                                    psB[m][:],
                                    lhsT=hidT[:, hc, P * m : P * (m + 1)],
                                    rhs=bpb4[:, hi, :],
                                    start=(hc == 0),
                                    stop=(hc == 31),
                                )
                    for m in range(4):
                        x3 = xp.tile([P, 512], bf16, tag="x3")
                        nc.vector.tensor_add(x3[:], psB[m][:], ysb[:, m, 512:1024])
                        ([nc.sync, nc.scalar][m % 2]).dma_start(
                            out=yout[P * m : P * (m + 1), 512:1024], in_=x3[:]
                        )

    nc.compile()
    _BUILD_CACHE["nc"] = nc
    return nc


def _local_rows(p):
    rows = []
    for t in range(4):
        g = 3 - t
        rows.extend(256 * g + 2 * j + p for j in range(P))
    return np.array(rows, dtype=np.int64)


def kernel(
    x,
    causal_mask,
    ln1_g,
    ln1_b,
    ln2_g,
    ln2_b,
    w_in,
    b_in,
    w_out,
    b_out,
    w_fc,
    b_fc,
    w_proj,
    b_proj,
):
    global LAST_RESULTS
    import ml_dtypes

    from concourse import bass_utils

    bf = ml_dtypes.bfloat16
    f8 = ml_dtypes.float8_e4m3
    x = np.asarray(x, np.float32)
    causal_mask = np.asarray(causal_mask, np.float32)
    f32 = lambda a: np.ascontiguousarray(np.asarray(a, np.float32))
    ln1_g, ln1_b, ln2_g, ln2_b = map(f32, (ln1_g, ln1_b, ln2_g, ln2_b))
    w_in, b_in, w_out, b_out = map(f32, (w_in, b_in, w_out, b_out))
    w_fc, b_fc, w_proj, b_proj = map(f32, (w_fc, b_fc, w_proj, b_proj))

    wq, wk, wv = w_in[:E], w_in[E : 2 * E], w_in[2 * E :]
    bq, bk, bv = b_in[:E], b_in[E : 2 * E], b_in[2 * E :]
    scale = 1.0 / math.sqrt(DH)

    wq2 = wq * ln1_g[None, :]          # 1/8 applied in the on-device descale
    bq2 = (wq @ ln1_b + bq) * scale
    wk2 = wk * ln1_g[None, :]
    bk2 = wk @ ln1_b + bk
    wv2 = wv * ln1_g[None, :]
    bv2 = wv @ ln1_b + bv
    bvec = b_out + w_out @ bv2
    wf2 = w_fc * ln2_g[None, :]
    bfc2 = b_fc + w_fc @ ln2_b
    bvec2 = b_proj

    def c8(a):
        return np.clip(a * W8, -240.0, 240.0).astype(f8)

    def tile_w8(wT2):
        return np.ascontiguousarray(
            c8(wT2).T.reshape(8, P, E).transpose(1, 0, 2)
        )

    wq_sb = tile_w8(wq2)
    wk_sb = tile_w8(wk2)
    wv_sb = tile_w8(wv2)
    wo_sb = np.ascontiguousarray(
        w_out.T.reshape(8, P, E).transpose(1, 0, 2).astype(bf)
    )
    wf_sb = np.ascontiguousarray(
        wf2.reshape(8, 4, P, 8, P).transpose(0, 4, 1, 3, 2).astype(bf)
    )
    wpT = np.ascontiguousarray(w_proj.T)  # [FF, E]
    wpa_sb = np.ascontiguousarray(
        wpT[:, :512].reshape(8, 4, P, 512).transpose(0, 2, 1, 3).astype(bf)
    )
    wpb_sb = np.ascontiguousarray(
        wpT[:, 512:].reshape(8, 4, P, 512).transpose(0, 2, 1, 3).astype(bf)
    )
    bq_sb = np.ascontiguousarray(bq2.reshape(8, P).T)
    bk_sb = np.ascontiguousarray(bk2.reshape(8, P).T)
    bfc_sb = np.ascontiguousarray(bfc2.reshape(32, P).T)

    cm = np.maximum(causal_mask, NEG)

    in_maps = []
    rows_by_p = [_local_rows(0), _local_rows(1)]
    # pair-swap permutation: position 2m holds token 2m+p, 2m+1 holds 2m+1-p,
    # so local (parity-p) tokens sit at the even columns for every core.
    pi_by_p = []
    msk_by_p = []
    for p in range(2):
        pi = np.empty(S, np.int64)
        pi[0::2] = np.arange(0, S, 2) + p
        pi[1::2] = np.arange(1, S, 2) - p
        pi_by_p.append(pi)
        msk = np.empty((P, 4, 256), np.float32)
        for t in range(4):
            ctx = S - 256 * t
            rows_t = rows_by_p[p][P * t : P * (t + 1)]
            msk[:, t, :] = cm[np.ix_(rows_t, pi[ctx - 256 : ctx])]
        msk_by_p.append(msk.astype(ml_dtypes.bfloat16))

    shared = dict(
        wq_sb=wq_sb,
        wk_sb=wk_sb,
        wv_sb=wv_sb,
        wo_sb=wo_sb,
        wf_sb=wf_sb,
        wpa_sb=wpa_sb,
        wpb_sb=wpb_sb,
        bq_sb=bq_sb,
        bk_sb=bk_sb,
        bfc_sb=bfc_sb,
        bvec2=np.ascontiguousarray(bvec2),
    )
    for c in range(NCORES):
        b, p = c // 2, c % 2
        m = dict(shared)
        xb = np.ascontiguousarray(x[b][pi_by_p[p]])
        xloc = np.ascontiguousarray(x[b][rows_by_p[p]])

        def pmaj(a, nt):
            # [nt*128, E] row-major -> [128, nt*E] partition-major tiles
            return np.ascontiguousarray(
                a.reshape(nt, P, E).transpose(1, 0, 2).reshape(P, nt * E)
            )

        m["x_glob_bf"] = pmaj(xb, 8).astype(bf)
        m["x_res_bf"] = pmaj(xloc + bvec[None, :], 4).astype(bf)
        m["msk"] = msk_by_p[p]
        in_maps.append(m)

    nc = _build()
    trace = bool(os.environ.get("KERNEL_TRACE"))
    res = bass_utils.run_bass_kernel_spmd(
        nc, in_maps, list(range(NCORES)), trace=trace
    )
    LAST_RESULTS = res

    x_out = np.empty((B, S, E), np.float32)
    att_w = np.empty((B, S, S), np.float32)
    for c in range(NCORES):
        b, p = c // 2, c % 2
        rows = rows_by_p[p]
        x_out[b][rows] = np.asarray(res.results[c]["y_out"]).astype(np.float32)
        aw = np.asarray(res.results[c]["aw_out"]).astype(np.float32) * (1.0 / H)
        for t in range(4):
            aw[P * t : P * (t + 1), S - 256 * t :] = 0.0
        att_w[b][np.ix_(rows, pi_by_p[p])] = aw
    return (x_out, att_w)

